# revision 23
# baseline (speedup 1.0000x reference)
"""Trainium2 Bass kernel for nn_Local_APro: affinity-based local propagation.

Reference computation (per image):
  F = img + 10
  aff_k = exp(-||F(p+delta_k) - F(p)||^2 / zeta^2)   (5x5 window, zero pad)
  x0    = feat * mask
  repeat 20x:  x <- mask * (sum_k aff_k * x(p+delta_k)) / sum_k aff_k

Sharding: 8 shards = 4 images x 2 height-halves, 40-row halo, no cross-core
communication; contaminated halo rows are discarded at host gather.

This environment executes ~1 instruction per ~33us regardless of engine or
operand size, so the design minimizes INSTRUCTION COUNT:

 * Rows live in two 128-row panels (ext rows 0..127 and 104..231), stored
   side by side on the free axis (f = panel*388 + padded_col, 776 wide).
   Each panel evolves all of its 128 rows; panels exchange 12-row halos only
   every 6 iterations (2 small DMAs x 3) - exactly enough to keep each
   panel's owned rows (0..115 / 116..231) uncontaminated for 6 iterations.
 * x is kept as "xrep" [128p, 5 slabs, 776]: slab j holds x shifted by
   dr = j-2 rows (partition-shifted SBUF->SBUF DMA; compute engines cannot
   read non-quadrant partition offsets). 4 bulk DMAs per iteration.
 * One iteration = 2 DVE instructions: a tensor_tensor with a 4D
   overlapping-window AP (prod[p,j,dw,dr] = aff[p,j,k]*xrep[p,dr,j+dw],
   tap k = dw*5+dr laid out contiguously), and a tensor_reduce(XY) over the
   25 taps writing new x directly into the next xrep's center slab.
 * aff is pre-normalized (aff * mask / sumz) and ZEROED at the 4 seam-pad
   columns, so the reduce rewrites pads with 0 and no per-iteration
   masking, normalization, or pad fixup is needed.
 * The host fills image pads with -10 (raw space), which makes the +10
   shift of the reference exact without any on-device work.
"""

import os
import sys

import numpy as np

_REPO = "/opt/trn_rl_repo"
try:
    import concourse.bass  # noqa: F401
except Exception:
    if os.path.isdir(_REPO) and _REPO not in sys.path:
        sys.path.insert(0, _REPO)

import concourse.bacc as bacc
import concourse.mybir as mybir
from concourse.bass_types import AP
from concourse.bass_utils import run_bass_kernel_spmd

K = 5
ZETA = 0.15
NUM_ITER = 20
B, C, H, W = 4, 3, 384, 384
HALF = 192            # kept output rows per shard
HALO = 2 * NUM_ITER   # 40 contaminated rows next to the cut
EXT = HALF + HALO     # 232 rows computed per shard
PB = 236              # padded slab rows (2 + EXT + 2)
WP = W + 4            # padded width 388
FW = 2 * WP           # flattened free width (both panels) = 776
FJ = FW - 4           # output positions per partition = 772
PSHIFT = 104          # panel B base ext row (B owns local rows 12..127)
REFRESH = 6           # halo-exchange period in iterations
F32 = mybir.dt.float32
F16 = mybir.dt.float16

_CACHE: dict = {}


def _ap(t, off, dims):
    """Raw AP on sbuf tensor handle t: dims = [[step_elems, count], ...]."""
    return AP(t.ap().tensor, off, [list(d) for d in dims])


def _build():
    nc = bacc.Bacc("TRN2", target_bir_lowering=False, debug=False)
    img_d = nc.dram_tensor("img", [PB, WP, C], F32, kind="ExternalInput")
    feat_d = nc.dram_tensor("feat", [EXT, W], F32, kind="ExternalInput")
    mask_d = nc.dram_tensor("mask", [EXT, W], F32, kind="ExternalInput")
    out_d = nc.dram_tensor("out", [EXT, W], F32, kind="ExternalOutput")

    NZ = -1.0 / (ZETA * ZETA)
    Exp = mybir.ActivationFunctionType.Exp
    MUL = mybir.AluOpType.mult
    ADD = mybir.AluOpType.add
    SUB = mybir.AluOpType.subtract
    X = mybir.AxisListType.X
    XY = mybir.AxisListType.XY

    XR = 5 * FW           # xrep pitch [128, 5, 776]
    AF = FJ * 25          # aff pitch [128, 772, 25]
    IM = FW * 3           # img pitch [128, 776, 3] (c minor)
    DD = FJ * 5 * 3       # dd pitch [128, 772, 5, 3] (fp16)
    SS = FJ * 5           # ssd pitch [128, 772, 5]
    SL2 = 2 * FW          # xrep center-slab offset

    n_it = NUM_ITER * int(os.environ.get("BASS_KERNEL_REPEAT", "1"))

    with (
        nc.sbuf_tensor([128, FJ, 25], F32) as aff,
        nc.sbuf_tensor([128, 5, FW], F32) as xr0,
        nc.sbuf_tensor([128, 5, FW], F32) as xr1,
        nc.sbuf_tensor([128, FJ], F32) as rn,
        nc.sbuf_tensor([128, FJ], F32) as msk,
        nc.sbuf_tensor([128, FJ, 25], F16) as prod,
        nc.semaphore() as dsem,
        nc.semaphore() as vsem,
        nc.semaphore() as ssem,
        nc.Block() as block,
    ):
        xr = [xr0, xr1]
        d = [0]
        v = [0]
        s_ = [0]
        sync_prog = []
        vec_prog = []
        act_prog = []

        def dma(dst, src):
            def f(eng, _d=dst, _s=src):
                eng.dma_start(_d, _s).then_inc(dsem, 16)
            d[0] += 16
            sync_prog.append(f)

        def dwait(val):
            sync_prog.append(lambda eng, _v=val: eng.wait_ge(dsem, _v))

        def swait_v(val):
            sync_prog.append(lambda eng, _v=val: eng.wait_ge(vsem, _v))

        def vec(op):
            def f(eng, _op=op):
                _op().then_inc(vsem, 1)
            v[0] += 1
            vec_prog.append(f)

        def vwait_d(val):
            vec_prog.append(lambda eng, _v=val: eng.wait_ge(dsem, _v))

        def vwait_s(val):
            vec_prog.append(lambda eng, _v=val: eng.wait_ge(ssem, _v))

        def act(op):
            def f(eng, _op=op):
                _op().then_inc(ssem, 1)
            s_[0] += 1
            act_prog.append(f)

        def awaits_v(val):
            act_prog.append(lambda eng, _v=val: eng.wait_ge(vsem, _v))

        # ---------------- schedule ----------------
        vec(lambda: nc.vector.memset(xr0.ap(), 0.0))
        vec(lambda: nc.vector.memset(xr1.ap(), 0.0))
        v_memset = v[0]

        # feat -> xr0 center slab interior (both panels, one DMA); mask load
        swait_v(v_memset)
        dma(_ap(xr0, SL2 + 2, [[XR, 128], [WP, 2], [1, W]]),
            AP(feat_d.ap().tensor, 0, [[W, 128], [PSHIFT * W, 2], [1, W]]))
        dma(_ap(msk, 0, [[FJ, 128], [WP, 2], [1, W]]),
            AP(mask_d.ap().tensor, 0, [[W, 128], [PSHIFT * W, 2], [1, W]]))
        d_init = d[0]

        x0 = _ap(xr0, SL2 + 2, [[XR, 128], [WP, 2], [1, W]])
        mski = _ap(msk, 0, [[FJ, 128], [WP, 2], [1, W]])
        vwait_d(d_init)
        vec(lambda: nc.vector.tensor_tensor(out=x0, in0=x0, in1=mski, op=MUL))
        v_x0 = v[0]

        # ---- affinity precompute ----
        with (
            nc.sbuf_tensor([128, FW, 3], F32) as img0,
            nc.sbuf_tensor([128, FW, 3], F32) as imgd,
            nc.sbuf_tensor([128, FJ, 5, 3], F16) as dd,
            nc.sbuf_tensor([128, FJ, 5], F32) as ssd,
        ):
            def img_load(t, dri):
                # slab row for (panel s, local p, shift dri) = s*104 + p + dri
                # img_d is host-prepared channel-minor [PB, WP, 3]
                for s in range(2):
                    dma(_ap(t, s * WP * 3, [[IM, 128], [1, WP * 3]]),
                        AP(img_d.ap().tensor, (dri + s * PSHIFT) * WP * 3,
                           [[WP * 3, 128], [1, WP * 3]]))

            img_load(img0, 2)  # dr = 0

            ddw = _ap(dd, 0, [[DD, 128], [15, FJ], [3, 5], [1, 3]])
            ddf = _ap(dd, 0, [[DD, 128], [1, FJ * 15]])
            ssdf = _ap(ssd, 0, [[SS, 128], [1, FJ * 5]])
            win0 = [[IM, 128], [3, FJ], [3, 5], [1, 3]]
            ctr = _ap(img0, 6, [[IM, 128], [3, FJ], [0, 5], [1, 3]])

            groups = [0, 1, 3, 4, 2]
            v_sub_prev = None
            s_exp_prev = None
            for gi, dri in enumerate(groups):
                if dri != 2:
                    if gi > 0:
                        swait_v(v_sub_prev)
                    img_load(imgd, dri)
                    gimg = imgd
                else:
                    gimg = img0
                d_g = d[0]

                win = _ap(gimg, 0, win0)
                vwait_d(d_g)
                if s_exp_prev is not None:
                    vwait_s(s_exp_prev)  # ssd reuse WAR
                vec(lambda _w=win: nc.vector.tensor_tensor(
                    out=ddw, in0=_w, in1=ctr, op=SUB))
                v_sub_prev = v[0]
                vec(lambda: nc.vector.tensor_tensor(
                    out=ddf, in0=ddf, in1=ddf, op=MUL))
                vec(lambda: nc.vector.tensor_reduce(
                    out=ssdf, in_=ddw, axis=X, op=ADD))
                v_csum = v[0]

                adst = _ap(aff, dri, [[AF, 128], [25, FJ], [5, 5]])
                sin = _ap(ssd, 0, [[SS, 128], [5, FJ], [1, 5]])
                awaits_v(v_csum)
                act(lambda _a=adst: nc.scalar.activation(
                    out=_a, in_=sin, func=Exp, scale=NZ))
                s_exp_prev = s_[0]

            # sumz -> rn -> normalize aff -> zero seam-pad columns
            aflat = _ap(aff, 0, [[AF, 128], [25, FJ], [1, 25]])
            vwait_s(s_exp_prev)
            vec(lambda: nc.vector.tensor_reduce(
                out=_ap(rn, 0, [[FJ, 128], [1, FJ]]),
                in_=aflat, axis=X, op=ADD))
            vec(lambda: nc.vector.reciprocal(rn.ap(), rn.ap()))
            vec(lambda: nc.vector.tensor_tensor(
                out=rn.ap(), in0=rn.ap(), in1=msk.ap(), op=MUL))
            rbc = _ap(rn, 0, [[FJ, 128], [1, FJ], [0, 25]])
            vec(lambda: nc.vector.tensor_tensor(
                out=aflat, in0=aflat, in1=rbc, op=MUL))
            vec(lambda: nc.vector.memset(
                _ap(aff, W * 25, [[AF, 128], [1, 100]]), 0.0))
            # (aff rows j in [384,388) = the 4 seam-pad output positions)

        # ---- bulk slab refresh ----
        def bulks(t):
            for dri in (0, 1, 3, 4):
                sh = dri - 2
                lo, hi = max(0, -sh), min(128, 128 - sh)
                dma(_ap(t, dri * FW + lo * XR, [[XR, hi - lo], [1, FW]]),
                    _ap(t, SL2 + (lo + sh) * XR, [[XR, hi - lo], [1, FW]]))

        swait_v(v_x0)
        bulks(xr0)
        d_bulk = d[0]

        # ---- iterations ----
        for t in range(n_it):
            xin, xout = xr[t % 2], xr[(t + 1) % 2]
            in1 = _ap(xin, 0, [[XR, 128], [1, FJ], [1, 5], [FW, 5]])
            in0 = _ap(aff, 0, [[AF, 128], [25, FJ], [5, 5], [1, 5]])
            po = _ap(prod, 0, [[AF, 128], [25, FJ], [5, 5], [1, 5]])
            ro = _ap(xout, SL2 + 2, [[XR, 128], [1, FJ]])
            pi = _ap(prod, 0, [[AF, 128], [25, FJ], [5, 5], [1, 5]])
            vwait_d(d_bulk)
            vec(lambda _a=in0, _b=in1, _o=po: nc.vector.tensor_tensor(
                out=_o, in0=_a, in1=_b, op=MUL))
            vec(lambda _i=pi, _o=ro: nc.vector.tensor_reduce(
                out=_o, in_=_i, axis=XY, op=ADD))
            v_red = v[0]

            if t != n_it - 1:
                swait_v(v_red)
                if (t + 1) % REFRESH == 0:
                    # halo exchange on xout center slab
                    dma(_ap(xout, SL2 + 116 * XR, [[XR, 12], [1, WP]]),
                        _ap(xout, SL2 + WP + 12 * XR, [[XR, 12], [1, WP]]))
                    dma(_ap(xout, SL2 + WP, [[XR, 12], [1, WP]]),
                        _ap(xout, SL2 + 104 * XR, [[XR, 12], [1, WP]]))
                    dwait(d[0])
                bulks(xout)
                d_bulk = d[0]

        # ---- output ----
        v_fin = v[0]
        xf = xr[n_it % 2]
        swait_v(v_fin)
        dma(AP(out_d.ap().tensor, 0, [[W, 116], [1, W]]),
            _ap(xf, SL2 + 2, [[XR, 116], [1, W]]))
        dma(AP(out_d.ap().tensor, 116 * W, [[W, 116], [1, W]]),
            _ap(xf, SL2 + WP + 2 + 12 * XR, [[XR, 116], [1, W]]))

        # ---------------- emit ----------------
        @block.sync
        def _(eng):
            for f in sync_prog:
                f(eng)

        @block.vector
        def _(eng):
            for f in vec_prog:
                f(eng)

        @block.scalar
        def _(eng):
            for f in act_prog:
                f(eng)

    nc.compile()
    return nc


def _program():
    if "nc" not in _CACHE:
        _CACHE["nc"] = _build()
    return _CACHE["nc"]


def kernel(img, feat, masked_box):
    img = np.asarray(img, np.float32)
    feat = np.asarray(feat, np.float32)
    mask = np.asarray(masked_box, np.float32)

    in_maps = []
    for core in range(8):
        b, half = core // 2, core % 2
        e0 = 0 if half == 0 else H - EXT
        slab = np.full((PB, WP, C), -10.0, np.float32)
        r0 = e0 - 2
        lo, hi = max(r0, 0), min(r0 + PB, H)
        slab[lo - r0:hi - r0, 2:386, :] = img[b, :, lo:hi, :].transpose(1, 2, 0)
        in_maps.append({
            "img": slab,
            "feat": np.ascontiguousarray(feat[b, e0:e0 + EXT, :]),
            "mask": np.ascontiguousarray(mask[b, e0:e0 + EXT, :]),
        })

    res = run_bass_kernel_spmd(_program(), in_maps, core_ids=list(range(8)))
    _CACHE["last_results"] = res

    out = np.empty((B, H, W), np.float32)
    for core in range(8):
        b, half = core // 2, core % 2
        r = res.results[core]["out"]
        if half == 0:
            out[b, :HALF] = r[:HALF]
        else:
            out[b, HALF:] = r[HALO:]
    return out
